# revision 4
# baseline (speedup 1.0000x reference)
"""Trainium2 Bass kernel for a 16-head causal attention block.

Problem: B=4, S=2048, D_MODEL=2048, N_HEADS=16, D_HEAD=128, fp32 I/O.

Sharding (8 cores): core c handles batch b = c//2 and head-group g = c%2
(8 heads each).  Each core computes its heads' attention and the partial
output projection (sum over its 8 heads) for its batch; the host sums the
two head-group partials per batch and adds the output bias.  No on-device
collectives needed.

Per-core dataflow (all matmuls bf16 operands, fp32 PSUM accumulation):
  xT resident in SBUF as [128(m_in), 16(m_tile), 2048(seq)]
  v_nat[k, hd]   = x @ Vw (+bias)         -> [128(k_in), 16(k_tile), 1024]
  per head h:
    qT[d, p]     = (Qw_h/sqrt(dh))^T-ish via lhsT=Qw tiles, rhs=xT (+bias)
    kT[d, p]     = same with Kw_h
    per q-block j (512 wide), k-tile i (128 wide, i <= 4j+3):
      ST[k,q]    = kT_tile^T-contract qT block      (PE, 1 matmul)
      PT         = exp(ST)                          (ACT, bf16 out)
      PT        *= causal mask  (diagonal tiles)    (DVE)
      acc       += PT                               (DVE, fp32)
      attnT     += v_tile^T-contract PT             (PE, PSUM accum)
    denom[1,q]   = ones^T-contract acc              (PE)
    recip        = approx 1/denom                   (DVE)
    recipB       = broadcast to 128 partitions      (GPSIMD)
    attnT_all    = attnT * recipB  (bf16)           (DVE)
  out[p, m]      = sum_h attnT_all_h^T-contract Ow_h   (PE) -> fp32 -> DRAM
"""

import math
import sys

import numpy as np
import ml_dtypes

for _p in ("/opt/trn_rl_repo",):
    if _p not in sys.path:
        sys.path.insert(0, _p)

BF16 = ml_dtypes.bfloat16

S_FULL = 2048
D_FULL = 2048
NH_LOC = 8  # heads per core
DH = 128
QB = 512  # q block width
N_CORES = 8


def build_program(seq=S_FULL, d_model=D_FULL, n_heads=NH_LOC):
    import concourse.tile as tile
    from concourse import bacc, mybir

    f32 = mybir.dt.float32
    bf16 = mybir.dt.bfloat16
    AF = mybir.ActivationFunctionType

    nt = d_model // 128  # contraction (d_model) tiles
    npt = seq // 128  # seq tiles (p / k)
    nqb = seq // QB  # q blocks
    kt_per_qb = QB // 128  # 4
    nhd = n_heads * DH  # concatenated head width
    nblk = nhd // 512  # 512-wide chunks of (h, d)

    nc = bacc.Bacc(
        "TRN2", target_bir_lowering=False, debug=False, enable_asserts=False
    )

    xt_d = nc.dram_tensor("xt", [128, nt, seq], bf16, kind="ExternalInput").ap()
    qw_d = nc.dram_tensor("qw", [n_heads, 128, nt, 128], bf16, kind="ExternalInput").ap()
    kw_d = nc.dram_tensor("kw", [n_heads, 128, nt, 128], bf16, kind="ExternalInput").ap()
    vw_d = nc.dram_tensor("vw", [128, nt, nhd], bf16, kind="ExternalInput").ap()
    ow_d = nc.dram_tensor("ow", [n_heads, 128, d_model], bf16, kind="ExternalInput").ap()
    qb_d = nc.dram_tensor("qb", [128, n_heads], f32, kind="ExternalInput").ap()
    kb_d = nc.dram_tensor("kb", [128, n_heads], f32, kind="ExternalInput").ap()
    vb_d = nc.dram_tensor("vb", [128, nhd], f32, kind="ExternalInput").ap()
    ones_d = nc.dram_tensor("ones", [128, 1], f32, kind="ExternalInput").ap()
    mask_d = nc.dram_tensor("mask", [128, 896], bf16, kind="ExternalInput").ap()
    out_d = nc.dram_tensor("out", [seq, d_model], f32, kind="ExternalOutput").ap()

    from concourse import library_config

    with tile.TileContext(nc) as tc:
        nc.gpsimd.load_library(library_config.attn)
        # PSUM pools: 2 + 3 + 2 + 1 = 8 banks
        mm_ps = tc.alloc_tile_pool(name="mmps", bufs=2, space="PSUM")
        st_ps = tc.alloc_tile_pool(name="stps", bufs=3, space="PSUM")
        pv_ps = tc.alloc_tile_pool(name="pvps", bufs=2, space="PSUM")
        dn_ps = tc.alloc_tile_pool(name="dnps", bufs=1, space="PSUM")

        consts = tc.alloc_tile_pool(name="consts", bufs=1)
        xt_pool = tc.alloc_tile_pool(name="xtp", bufs=1)
        vall_pool = tc.alloc_tile_pool(name="vallp", bufs=1)
        att_pool = tc.alloc_tile_pool(name="attp", bufs=1, side="right")

        mask_sb = consts.tile([128, 896], bf16)
        nc.sync.dma_start(mask_sb, mask_d)
        ones_sb = consts.tile([128, 1], f32)
        nc.sync.dma_start(ones_sb, ones_d)
        qb_sb = consts.tile([128, n_heads], f32)
        nc.sync.dma_start(qb_sb, qb_d)
        kb_sb = consts.tile([128, n_heads], f32)
        nc.sync.dma_start(kb_sb, kb_d)
        vb_sb = consts.tile([128, nhd], f32)
        nc.sync.dma_start(vb_sb, vb_d)

        xt = xt_pool.tile([128, nt, seq], bf16)
        for t in range(nt):
            nc.sync.dma_start(xt[:, t, :], xt_d[:, t, :])

        vall = vall_pool.tile([128, npt, nhd], bf16)
        attall = att_pool.tile([128, n_heads, seq], bf16)

        # ---------------- phase 1: v_nat for all heads ----------------
        vw_pool = tc.alloc_tile_pool(name="vwp", bufs=1)
        vw = vw_pool.tile([128, nt, nhd], bf16)
        for t in range(nt):
            nc.sync.dma_start(vw[:, t, :], vw_d[:, t, :])
        for p_i in range(npt):
            for blk in range(nblk):
                vps = mm_ps.tile([128, 512], f32, tag="mm", name=f"vps_{p_i}_{blk}")
                for m in range(nt):
                    nc.tensor.matmul(
                        vps,
                        xt[:, m, p_i * 128 : (p_i + 1) * 128],
                        vw[:, m, blk * 512 : (blk + 1) * 512],
                        start=(m == 0),
                        stop=(m == nt - 1),
                    )
                nc.vector.tensor_add(
                    vall[:, p_i, blk * 512 : (blk + 1) * 512],
                    vps,
                    vb_sb[:, blk * 512 : (blk + 1) * 512],
                )
        vw_pool.release()

        # ---------------- phase 2: per-head attention ----------------
        qk_pool = tc.alloc_tile_pool(name="qkp", bufs=2)
        w_pool = tc.alloc_tile_pool(name="wp", bufs=2)
        pt_pool = tc.alloc_tile_pool(name="ptp", bufs=6)
        acc_pool = tc.alloc_tile_pool(name="accp", bufs=2)
        rc_pool = tc.alloc_tile_pool(name="rcp", bufs=2)
        rb_pool = tc.alloc_tile_pool(name="rbp", bufs=2)

        for h in range(n_heads):
            wq = w_pool.tile([128, nt, 128], bf16, tag="wq", name=f"wq_{h}")
            nc.sync.dma_start(wq, qw_d[h])
            wk = w_pool.tile([128, nt, 128], bf16, tag="wk", name=f"wk_{h}")
            nc.sync.dma_start(wk, kw_d[h])
            qT = qk_pool.tile([128, seq], bf16, tag="qT", name=f"qT_{h}")
            kT = qk_pool.tile([128, seq], bf16, tag="kT", name=f"kT_{h}")
            for pb in range(seq // 512):
                qps = mm_ps.tile([128, 512], f32, tag="mm", name=f"qps_{h}_{pb}")
                for m in range(nt):
                    nc.tensor.matmul(
                        qps,
                        wq[:, m, :],
                        xt[:, m, pb * 512 : (pb + 1) * 512],
                        start=(m == 0),
                        stop=(m == nt - 1),
                    )
                nc.scalar.activation(
                    qT[:, pb * 512 : (pb + 1) * 512],
                    qps,
                    AF.Identity,
                    bias=qb_sb[:, h : h + 1],
                )
                kps = mm_ps.tile([128, 512], f32, tag="mm", name=f"kps_{h}_{pb}")
                for m in range(nt):
                    nc.tensor.matmul(
                        kps,
                        wk[:, m, :],
                        xt[:, m, pb * 512 : (pb + 1) * 512],
                        start=(m == 0),
                        stop=(m == nt - 1),
                    )
                nc.scalar.activation(
                    kT[:, pb * 512 : (pb + 1) * 512],
                    kps,
                    AF.Identity,
                    bias=kb_sb[:, h : h + 1],
                )
            for j in range(nqb):
                nk = (j + 1) * kt_per_qb
                aps = pv_ps.tile([128, 512], f32, tag="pv", name=f"aps_{h}_{j}")
                acc = acc_pool.tile([128, 512], f32, tag="acc", name=f"acc_{h}_{j}")
                for i in range(nk):
                    stp = st_ps.tile([128, 512], f32, tag="st", name=f"stp_{h}_{j}_{i}")
                    nc.tensor.matmul(
                        stp,
                        kT[:, i * 128 : (i + 1) * 128],
                        qT[:, j * 512 : (j + 1) * 512],
                        start=True,
                        stop=True,
                    )
                    ptile = pt_pool.tile(
                        [128, 512], bf16, tag="pt", name=f"pt_{h}_{j}_{i}"
                    )
                    nc.scalar.activation(ptile, stp, AF.Exp)
                    if i >= kt_per_qb * j:
                        s = 128 * (i - kt_per_qb * j)
                        nc.vector.tensor_mul(
                            ptile, ptile, mask_sb[:, 384 - s : 896 - s]
                        )
                    if i == 0:
                        nc.vector.tensor_copy(acc, ptile)
                    else:
                        nc.vector.tensor_add(acc, acc, ptile)
                    nc.tensor.matmul(
                        aps,
                        vall[:, i, h * 128 : (h + 1) * 128],
                        ptile,
                        start=(i == 0),
                        stop=(i == nk - 1),
                    )
                dn = dn_ps.tile([1, 512], f32, tag="dn", name=f"dn_{h}_{j}")
                nc.tensor.matmul(dn, ones_sb, acc, start=True, stop=True)
                rc = rc_pool.tile([1, 512], f32, tag="rc", name=f"rc_{h}_{j}")
                nc.vector.reciprocal_approx_fast(rc, dn)
                rb = rb_pool.tile([128, 512], f32, tag="rb", name=f"rb_{h}_{j}")
                nc.gpsimd.partition_broadcast(rb, rc)
                nc.vector.tensor_mul(
                    attall[:, h, j * 512 : (j + 1) * 512], aps, rb
                )
        rb_pool.release()
        rc_pool.release()
        acc_pool.release()
        pt_pool.release()
        w_pool.release()
        qk_pool.release()
        vall_pool.release()
        xt_pool.release()

        # ---------------- phase 3: output projection ----------------
        ow_pool = tc.alloc_tile_pool(name="owp", bufs=2 * n_heads, side="right")
        osb_pool = tc.alloc_tile_pool(name="osbp", bufs=4, side="right")
        for mb in range(d_model // 512):
            ows = []
            for h in range(n_heads):
                owt = ow_pool.tile([128, 512], bf16, tag="ow", name=f"ow_{mb}_{h}")
                nc.sync.dma_start(owt, ow_d[h][:, mb * 512 : (mb + 1) * 512])
                ows.append(owt)
            for p_i in range(npt):
                ops = mm_ps.tile([128, 512], f32, tag="mm", name=f"ops_{mb}_{p_i}")
                for h in range(n_heads):
                    nc.tensor.matmul(
                        ops,
                        attall[:, h, p_i * 128 : (p_i + 1) * 128],
                        ows[h],
                        start=(h == 0),
                        stop=(h == n_heads - 1),
                    )
                osb = osb_pool.tile([128, 512], f32, tag="osb", name=f"osb_{mb}_{p_i}")
                nc.scalar.copy(osb, ops)
                nc.sync.dma_start(
                    out_d[p_i * 128 : (p_i + 1) * 128, mb * 512 : (mb + 1) * 512],
                    osb,
                )
        osb_pool.release()
        ow_pool.release()
        att_pool.release()
        consts.release()
        dn_ps.release()
        pv_ps.release()
        st_ps.release()
        mm_ps.release()
    nc.finalize()
    return nc


def make_core_inputs(x_b, Qw, Qb, Kw, Kb, Vw, Vb, Ow, seq, d_model, n_heads):
    """Host-side prep of one core's input map.

    x_b: [seq, d_model] fp32.  Qw/Kw/Vw: [n_heads, d_model, 128].
    Qb/Kb/Vb: [n_heads, 128].  Ow: [n_heads, 128, d_model].
    """
    nt = d_model // 128
    nhd = n_heads * DH
    scale = 1.0 / math.sqrt(DH)

    # xT as [128(m_in), nt, seq]
    xt = np.ascontiguousarray(
        x_b.T.reshape(nt, 128, seq).transpose(1, 0, 2).astype(BF16)
    )
    qw = np.ascontiguousarray(
        (Qw * scale).reshape(n_heads, nt, 128, 128).transpose(0, 2, 1, 3).astype(BF16)
    )
    kw = np.ascontiguousarray(
        Kw.reshape(n_heads, nt, 128, 128).transpose(0, 2, 1, 3).astype(BF16)
    )
    # v weights as [128(m_in), nt, (h d)]
    vw = np.ascontiguousarray(
        Vw.transpose(1, 0, 2)
        .reshape(d_model, nhd)
        .reshape(nt, 128, nhd)
        .transpose(1, 0, 2)
        .astype(BF16)
    )
    ow = np.ascontiguousarray(Ow.astype(BF16))
    qb = np.ascontiguousarray((Qb * scale).T.astype(np.float32))
    kb = np.ascontiguousarray(Kb.T.astype(np.float32))
    vb = np.ascontiguousarray(
        np.broadcast_to(Vb.reshape(1, nhd), (128, nhd)).astype(np.float32)
    )
    ones = np.ones((128, 1), np.float32)
    r = np.arange(128, dtype=np.int64)[:, None]
    u = np.arange(896, dtype=np.int64)[None, :]
    mask = (r <= u - 384).astype(BF16)
    return {
        "xt": xt,
        "qw": qw,
        "kw": kw,
        "vw": vw,
        "ow": ow,
        "qb": qb,
        "kb": kb,
        "vb": vb,
        "ones": ones,
        "mask": mask,
    }


_NC_CACHE = None


def kernel(**inputs):
    global _NC_CACHE
    from concourse.bass_utils import run_bass_kernel_spmd

    x = np.asarray(inputs["x"], np.float32)
    Q_w = np.asarray(inputs["Q_w"], np.float32)
    Q_b = np.asarray(inputs["Q_b"], np.float32)
    K_w = np.asarray(inputs["K_w"], np.float32)
    K_b = np.asarray(inputs["K_b"], np.float32)
    V_w = np.asarray(inputs["V_w"], np.float32)
    V_b = np.asarray(inputs["V_b"], np.float32)
    O_w = np.asarray(inputs["O_w"], np.float32)
    O_b = np.asarray(inputs["O_b"], np.float32)

    B, seq, d_model = x.shape

    if _NC_CACHE is None:
        _NC_CACHE = build_program(seq=seq, d_model=d_model, n_heads=NH_LOC)
    nc = _NC_CACHE

    in_maps = []
    for c in range(N_CORES):
        b = c // 2
        g = c % 2
        hs = slice(g * NH_LOC, (g + 1) * NH_LOC)
        in_maps.append(
            make_core_inputs(
                x[b], Q_w[hs], Q_b[hs], K_w[hs], K_b[hs], V_w[hs], V_b[hs],
                O_w[hs], seq, d_model, NH_LOC,
            )
        )

    res = run_bass_kernel_spmd(nc, in_maps, core_ids=list(range(N_CORES)))
    out = np.empty((B, seq, d_model), np.float32)
    for b in range(B):
        out[b] = res.results[2 * b]["out"] + res.results[2 * b + 1]["out"] + O_b[None, :]
    return out
